# revision 32
# baseline (speedup 1.0000x reference)
"""YOLO-detect head (1x1 conv + box decode) on 8 Trainium2 NeuronCores.

Data-parallel over batch: core b processes batch element b.

Per core, per level l (C channels, HW = ny*nx positions):
  p[hw, o] = sum_c x[c, hw] * w[o, c]      (o = a*89 + ch, a anchor, ch channel)
computed on the tensor engine as out = lhsT.T @ rhs with
  lhsT = x chunk  [K=128 channels, M=128 hw]  (stationary; fp8-e3m4 for levels
         0/1, fp16 for level 2 -- mixed fp8xfp16 matmul is legal on TRN2)
  rhs  = w.T chunk [K=128 channels, N=267]    (moving, fp16)
so the PSUM result is already [hw, 267] -- no on-chip transpose.

Decode (ACT engine is the critical resource: 1 elem/cycle/lane @1.2GHz,
~352cy fixed overhead per ACTIVATE):
  * ONE table set (exp_and_others: tanh+exp) for the whole kernel.
  * Per 4-tile PSUM group, ONE Tanh over all 89 cols writes t = tanh(0.5*p)
    directly as fp8-e3m4 into the big output tile og8.  sigmoid = 0.5*t+0.5
    is applied by the HOST during dequant (a scaled-fp8 codec).
  * wh: DVE stages the raw logits PSUM->SBUF per group (it is otherwise
    idle); ONE Exp per level at level end, then one anchor-multiply.
  * xy: derived per chunk from the fp8 tanh values with fused DVE
    scalar_tensor_tensor ops: xy = t*(stride/2) + (stride*(grid+0.5)).

Scheduling (what v2's trace taught):
  * Levels run L0, then L2 interleaved after L0's 8th group, then L1.
    L2's 32-matmul group fills the PE while ACT drains L0's tanh backlog;
    ending on L1 keeps the serial tail short.
  * Early loads are spread over three HWDGE queues (gsam on vector,
    wt0/wt2/x2 on scalar, x0 pieces + wt1/x1 on sync) so DGE setup times
    overlap and the first matmul starts ~2us sooner.
  * Stores ride nc.gpsimd (SWDGE) so a blocked store never stalls loads.

Error budget (tolerance 2e-2 of absmax~1132): fp8-e3m4 x-quant worst-cases
the level-1 wh channels at ~4e-3 (measured 1.7e-3 total); fp8 t-storage
adds ~7e-6; everything else is at the old fp16 level (~5e-4).
"""

import numpy as np
import ml_dtypes

import concourse.bacc as bacc
import concourse.mybir as mybir
import concourse.tile as tile
from concourse.bass_utils import run_bass_kernel_spmd

F32 = mybir.dt.float32
F16 = mybir.dt.float16
F8 = mybir.dt.float8e3   # e3m4: 4 mantissa bits, range +-15.5
F8E4 = mybir.dt.float8e4  # e4m3 (TRN flavor, max 240) -- DoubleRow needs it
NP_F8 = ml_dtypes.float8_e3m4
NP_F8E4 = ml_dtypes.float8_e4m3
AF = mybir.ActivationFunctionType
ALU = mybir.AluOpType
DR = mybir.MatmulPerfMode.DoubleRow

NCORES = 8
NA = 3          # anchors per level
NO = 89         # channels per anchor (80 classes + 5 + 4)
NCOL = NA * NO  # 267
GROUP = 4       # full 128-row hw tiles per PSUM group (4 banks, 2 in flight)
CHUNK = 16      # tiles per store chunk / per DVE xy-fixup op
WSCALE = 16.0   # pre-quant scale on fp8 w (keeps it out of denormals)
NWARM = 16      # dummy matmuls that trip the HAM clock gate early; 16 x
                # ~432ns cold = ~6.9us busy, covering a full aligned 4096cy
                # HAM window even at the worst free-running phase

LEVELS = [
    dict(C=256,  W=80, HW=6400, stride=8.0, xdt=F8, dr=False, wq=False,
         exp_defer=True,
         anchors=((10.0, 13.0), (16.0, 30.0), (33.0, 23.0))),
    # level 1 runs last: per-group exp keeps its tail short
    dict(C=512,  W=40, HW=1600, stride=16.0, xdt=F8, dr=False, wq=False,
         exp_defer=False,
         anchors=((30.0, 61.0), (62.0, 45.0), (59.0, 119.0))),
    dict(C=1024, W=20, HW=400,  stride=32.0, xdt=F16, dr=False, wq=False,
         exp_defer=False,
         anchors=((116.0, 90.0), (156.0, 198.0), (373.0, 326.0))),
]
for _L in LEVELS:
    _L["nt"] = (_L["HW"] + 127) // 128      # 50, 13, 4
    _L["HWp"] = _L["nt"] * 128              # padded positions (6400, 1664, 512)
    _L["psc"] = WSCALE if _L["wq"] else 1.0  # psum = psc * p

# (level, group-slice) segments in issue order.  L2's PE-heavy group sits
# mid-stream where ACT has backlog; L1 runs last (its final group is 1 tile
# so the tail store chain is tiny).  The PE's queue is in-order, so work
# must not be scheduled before its x data can possibly have arrived.
SCHEDULE = [(0, 0, 9), (2, 0, 1), (0, 9, 10), (2, 1, 2), (1, 0, 1),
            (0, 10, 12), (1, 1, 2), (0, 12, 13), (1, 2, 3), (0, 13, 14),
            (1, 3, 4)]


def _groups(nt, base=0):
    """[(t0, ntl)] covering tiles [base, base+nt) in GROUP-sized pieces."""
    out = []
    t0 = 0
    while t0 < nt:
        out.append((base + t0, min(GROUP, nt - t0)))
        t0 += GROUP
    return out


def _chunks(nt):
    """[(s0, snt)] store/fixup chunks, group-aligned, trailing runt merged."""
    out = []
    s = 0
    while s < nt:
        e = min(s + CHUNK, nt)
        if nt - e < GROUP:
            e = nt
        out.append((s, e - s))
        s = e
    return out


def _build_program(use_bias: bool):
    nc = bacc.Bacc("TRN2", target_bir_lowering=False, debug=False)

    # gs' and am, both [128, NA, 2, nt] fp16 per level, concatenated
    GSAM_COLS = sum(NA * 2 * L["nt"] * 2 for L in LEVELS)  # 804

    dram = {}
    for l, L in enumerate(LEVELS):
        KC = L["C"] // 128
        nt = L["nt"]
        wdt = F8E4 if L["wq"] else F16
        dram[f"x{l}"] = nc.dram_tensor(f"x{l}", (128, KC * L["HW"]), L["xdt"],
                                       kind="ExternalInput").ap()
        dram[f"wt{l}"] = nc.dram_tensor(f"wt{l}", (128, KC * NCOL), wdt,
                                        kind="ExternalInput").ap()
        # t=tanh(.5 p) for every col; partition p holds hw rows {t*128+p}
        dram[f"y8_{l}"] = nc.dram_tensor(f"y8_{l}", (128, NA, nt, NO), F8,
                                         kind="ExternalOutput").ap()
        # final xy (cols 0:2) and exp(p)*anchor (cols 2:4), c-major runs
        dram[f"yx_{l}"] = nc.dram_tensor(f"yx_{l}", (128, NA, 4, nt), F16,
                                         kind="ExternalOutput").ap()
        if use_bias:
            dram[f"b{l}"] = nc.dram_tensor(f"b{l}", (1, NCOL), F32,
                                           kind="ExternalInput").ap()
    dram["gsam"] = nc.dram_tensor("gsam", (128, GSAM_COLS), F16,
                                  kind="ExternalInput").ap()

    with tile.TileContext(nc) as tc:
        with tc.tile_pool(name="consts", bufs=1) as cpool, \
             tc.tile_pool(name="xbuf", bufs=1) as xpool, \
             tc.tile_pool(name="obuf", bufs=1) as opool, \
             tc.tile_pool(name="ps", bufs=2, space="PSUM") as pspool:

            ones_t = None
            if use_bias:
                ones_t = cpool.tile([1, 128], F16, tag="ones", name="ones")
                nc.vector.memset(ones_t[:, :], 1.0)

            # ---- loads, spread across HWDGE queues ----
            gsam_t = cpool.tile([128, GSAM_COLS], F16, tag="gsam",
                                name="gsamsb")
            nc.gpsimd.dma_start(out=gsam_t[:, :], in_=dram["gsam"][:, :])

            lvl = {}
            for l, L in enumerate(LEVELS):
                KC = L["C"] // 128
                wdt = F8E4 if L["wq"] else F16
                lvl[l] = dict(
                    wt=cpool.tile([128, KC, NCOL], wdt, tag=f"wt{l}",
                                  name=f"wt{l}sb"),
                    xk=xpool.tile([128, KC, L["HWp"]], L["xdt"], tag=f"x{l}",
                                  name=f"xk{l}"),
                    b_t=None,
                )
                if L["HWp"] != L["HW"]:
                    # pad cols feed the partial tile's matmul rows: zero them
                    nc.gpsimd.memset(lvl[l]["xk"][:, :, L["HW"]:L["HWp"]], 0.0)
                if use_bias:
                    lvl[l]["b_t"] = cpool.tile([1, NCOL], F32, tag=f"b{l}",
                                               name=f"bt{l}")
                    nc.gpsimd.dma_start(out=lvl[l]["b_t"][:, :],
                                        in_=dram[f"b{l}"][:, :])

            def xsrc(l):
                return dram[f"x{l}"].rearrange(
                    "p (k w) -> p k w", k=LEVELS[l]["C"] // 128)

            def wsrc(l):
                return dram[f"wt{l}"].rearrange(
                    "p (k o) -> p k o", k=LEVELS[l]["C"] // 128)

            # ALL loads ride the sync queue in strict need-order: a second
            # load queue just steals DMA-engine service from the critical
            # x0 pieces (measured: wt1+wt2 on scalar delayed x0p1 by ~3us)
            nc.sync.dma_start(out=lvl[0]["wt"][:, :, :], in_=wsrc(0))
            for (c0, c1) in ((0, 1024), (1024, 3072), (3072, 6400)):
                nc.sync.dma_start(out=lvl[0]["xk"][:, :, c0:c1],
                                  in_=xsrc(0)[:, :, c0:c1])
            nc.sync.dma_start(out=lvl[2]["wt"][:, :, :], in_=wsrc(2))
            nc.sync.dma_start(out=lvl[2]["xk"][:, :, 0:400], in_=xsrc(2))
            nc.sync.dma_start(out=lvl[1]["wt"][:, :, :], in_=wsrc(1))
            nc.sync.dma_start(out=lvl[1]["xk"][:, :, 0:1600], in_=xsrc(1))

            off = 0
            for l, L in enumerate(LEVELS):
                n6 = NA * 2 * L["nt"]
                lvl[l]["gs"] = gsam_t[:, off:off + n6].rearrange(
                    "p (a c t) -> p a c t", a=NA, c=2)
                off += n6
                lvl[l]["am"] = gsam_t[:, off:off + n6].rearrange(
                    "p (a c t) -> p a c t", a=NA, c=2)
                off += n6
                lvl[l]["og8"] = opool.tile([128, NA, L["nt"], NO], F8,
                                           tag=f"og8_{l}", name=f"og8_{l}")
                lvl[l]["oxw"] = opool.tile([128, NA, 4, L["nt"]], F16,
                                           tag=f"oxw{l}", name=f"oxw{l}")
                if L["exp_defer"]:
                    lvl[l]["pwh"] = opool.tile([128, NA, L["nt"], 2], F16,
                                               tag=f"pwh{l}", name=f"pwh{l}")
                lvl[l]["chunks"] = ([(0, 8), (8, 5)] if l == 1
                                    else _chunks(L["nt"]))
                lvl[l]["next_chunk"] = 0
                # L2 as two 2-tile groups: its 32-matmul block otherwise
                # starves ACT for ~3us.  L0 starts with two 2-tile groups so
                # the first tanh issues earlier while the PE is still cold.
                if l == 2:
                    lvl[l]["groups"] = [(0, 2), (2, 2)]
                elif l == 0:
                    lvl[l]["groups"] = ([(0, 2), (2, 2)]
                                        + _groups(L["nt"] - 4, base=4))
                else:
                    lvl[l]["groups"] = _groups(L["nt"])

            # ---- compute; stores via SWDGE (gpsimd) ----
            def emit_segment(l, g0, g1):
                L = LEVELS[l]
                KC = L["C"] // 128
                stride, nt, psc = L["stride"], L["nt"], L["psc"]
                d = lvl[l]
                wt_t, xk, b_t = d["wt"], d["xk"], d["b_t"]
                og8, oxw = d["og8"], d["oxw"]

                for (t0, ntl) in d["groups"][g0:g1]:
                    ps = pspool.tile([128, GROUP, 512], F32, tag="ps",
                                     name=f"ps{l}_{t0}")
                    psf = ps.rearrange("p g x -> p (g x)")
                    for i in range(ntl):
                        t = t0 + i
                        if L["dr"]:
                            # fp8 DoubleRow: k-pairs as [Ki, 2, free] APs
                            for kc in range(0, KC, 2):
                                nc.tensor.matmul(
                                    psf[:, i * 512:i * 512 + NCOL],
                                    lhsT=xk[:, kc:kc + 2,
                                            t * 128:(t + 1) * 128],
                                    rhs=wt_t[:, kc:kc + 2, :],
                                    start=(kc == 0),
                                    stop=(kc == KC - 2 and not use_bias),
                                    perf_mode=DR,
                                )
                        else:
                            for kc in range(KC):
                                nc.tensor.matmul(
                                    psf[:, i * 512:i * 512 + NCOL],
                                    lhsT=xk[:, kc, t * 128:(t + 1) * 128],
                                    rhs=wt_t[:, kc, :],
                                    start=(kc == 0),
                                    stop=(kc == KC - 1 and not use_bias),
                                )
                        if use_bias:
                            nc.tensor.matmul(
                                psf[:, i * 512:i * 512 + NCOL],
                                lhsT=ones_t[:, :],
                                rhs=b_t[:, :],
                                start=False,
                                stop=True,
                            )

                    # psum viewed anchor-major: [p, a, g, c]; psum = psc * p
                    ps_a = ps[:, 0:ntl, 0:NCOL].rearrange(
                        "p g (a c) -> p a g c", a=NA)
                    # t = tanh(0.5*p) straight to fp8; host decodes 0.5t+0.5
                    nc.scalar.activation(og8[:, :, t0:t0 + ntl, :], ps_a,
                                         AF.Tanh, scale=0.5 / psc)
                    if L["exp_defer"]:
                        # stage wh logits for the per-level batched Exp
                        nc.vector.tensor_copy(d["pwh"][:, :, t0:t0 + ntl, :],
                                              ps_a[:, :, :, 2:4])
                    else:
                        nc.scalar.activation(
                            oxw[:, :, 2:4, t0:t0 + ntl],
                            ps_a[:, :, :, 2:4].transpose([0, 1, 3, 2]),
                            AF.Exp, scale=1.0 / psc)

                    while (d["next_chunk"] < len(d["chunks"])
                           and d["chunks"][d["next_chunk"]][0]
                           + d["chunks"][d["next_chunk"]][1] <= t0 + ntl):
                        s0, snt = d["chunks"][d["next_chunk"]]
                        # xy = t*(stride/2) + stride*(grid+0.5)
                        # (fused stt; one per anchor -- stt APs max 3D)
                        for a in range(NA):
                            nc.vector.scalar_tensor_tensor(
                                oxw[:, a, 0:2, s0:s0 + snt],
                                og8[:, a, s0:s0 + snt, 0:2]
                                .transpose([0, 2, 1]),
                                float(stride / 2),
                                d["gs"][:, a, :, s0:s0 + snt],
                                ALU.mult, ALU.add)
                        final = (l == 1 and d["next_chunk"]
                                 == len(d["chunks"]) - 1)
                        # route the kernel's last og8 store via the (idle by
                        # then) sync HWDGE so it issues in parallel with the
                        # oxw store below
                        eng = nc.sync if final else nc.gpsimd
                        eng.dma_start(
                            out=dram[f"y8_{l}"][:, :, s0:s0 + snt, :],
                            in_=og8[:, :, s0:s0 + snt, :])
                        d["next_chunk"] += 1

                if g1 >= len(d["groups"]):  # level finished
                    assert d["next_chunk"] == len(d["chunks"])
                    if L["exp_defer"]:
                        # wh = exp(p): one batched Exp off the staged logits
                        nc.scalar.activation(
                            oxw[:, :, 2:4, :],
                            d["pwh"][:, :, :, :].transpose([0, 1, 3, 2]),
                            AF.Exp, scale=1.0 / psc)
                    nc.vector.tensor_mul(oxw[:, :, 2:4, :],
                                         oxw[:, :, 2:4, :],
                                         d["am"][:, :, :, :])
                    eng = nc.scalar if l == 1 else nc.gpsimd
                    eng.dma_start(out=dram[f"yx_{l}"][:, :, :, :],
                                  in_=oxw[:, :, :, :])

            for (l, g0, g1) in SCHEDULE:
                emit_segment(l, g0, g1)
    nc.compile()
    return nc


_PROGS = {}


def _get_prog(use_bias: bool):
    if use_bias not in _PROGS:
        _PROGS[use_bias] = _build_program(use_bias)
    return _PROGS[use_bias]


def _host_gsam():
    """[gs'0|am0|gs'1|am1|gs'2|am2], each [128, NA, 2, nt] fp16 flattened.

    gs'[p, a, c, t] = stride*(grid_c(t*128+p) + 0.5); am[p, a, c, t] = A[a][c].
    """
    cols = []
    for L in LEVELS:
        HW, W, stride, nt = L["HW"], L["W"], L["stride"], L["nt"]
        hw = np.arange(nt * 128)
        gx = (hw % W).astype(np.float32)
        gy = (hw // W).astype(np.float32)
        g = np.stack([gx, gy], axis=0)          # (2, nt*128)
        gsp = (g + 0.5) * stride
        gsp[:, HW:] = 0.0
        # (2, nt, 128) -> [p, c, t]
        gsp = gsp.reshape(2, nt, 128).transpose(2, 0, 1)
        gs = np.broadcast_to(gsp[:, None], (128, NA, 2, nt))
        am = np.broadcast_to(
            np.asarray(L["anchors"], np.float32)[None, :, :, None],
            (128, NA, 2, nt))
        cols.append(gs.reshape(128, -1))
        cols.append(am.reshape(128, -1))
    return np.ascontiguousarray(
        np.concatenate(cols, axis=1).astype(np.float16))


_CONSTS = None


def _make_in_maps(xs, ws, bs, use_bias):
    global _CONSTS
    if _CONSTS is None:
        _CONSTS = _host_gsam()
    wts, xps = [], []
    for x, w, L in zip(xs, ws, LEVELS):
        KC = L["C"] // 128
        HW = L["HW"]
        npdt = {F8: NP_F8, F8E4: NP_F8E4, F16: np.float16}[L["xdt"]]
        wdt = NP_F8E4 if L["wq"] else np.float16
        # (C, NCOL) -> (128, KC*NCOL): row p col (k*NCOL+o) = w[o, k*128+p]
        wts.append(np.ascontiguousarray(
            (w.T * (WSCALE if L["wq"] else 1.0)).astype(wdt)
            .reshape(KC, 128, NCOL)
            .transpose(1, 0, 2).reshape(128, KC * NCOL)))
        # (B, C, H, W) -> (B, 128, KC*HW): row p col (k*HW+hw) = x[k*128+p, hw]
        xps.append(np.ascontiguousarray(
            x.reshape(NCORES, KC, 128, HW).astype(npdt)
            .transpose(0, 2, 1, 3).reshape(NCORES, 128, KC * HW)))
    in_maps = []
    for core in range(NCORES):
        im = {"gsam": _CONSTS}
        for l in range(len(LEVELS)):
            im[f"x{l}"] = xps[l][core]
            im[f"wt{l}"] = wts[l]
            if use_bias:
                im[f"b{l}"] = np.ascontiguousarray(
                    (bs[l] * LEVELS[l]["psc"]).reshape(1, NCOL)
                    .astype(np.float32))
        in_maps.append(im)
    return in_maps


def _assemble(results):
    """y8 (128,NA,nt,89) fp8 + yx (128,NA,4,nt) fp16 -> (NCORES,25200,89)."""
    out = np.empty((NCORES, 25200, NO), np.float32)
    for core in range(NCORES):
        parts = []
        for l, L in enumerate(LEVELS):
            HW, nt = L["HW"], L["nt"]
            t8 = results[core][f"y8_{l}"].astype(np.float32)
            # sigmoid = 0.5*t + 0.5 (fp8 codec dequant)
            y = t8 * 0.5 + 0.5
            y = y.transpose(1, 2, 0, 3).reshape(NA, nt * 128, NO)[:, :HW, :]
            xw = results[core][f"yx_{l}"].astype(np.float32)
            xw = xw.transpose(1, 3, 0, 2).reshape(NA, nt * 128, 4)[:, :HW, :]
            y[:, :, 0:4] = xw
            parts.append(y.reshape(NA * HW, NO))
        out[core] = np.concatenate(parts, axis=0)
    return out


def _run(x0, x1, x2, w0, b0, w1, b1, w2, b2, **spmd_kwargs):
    xs = [np.asarray(x, dtype=np.float32) for x in (x0, x1, x2)]
    ws = [np.asarray(w, dtype=np.float32) for w in (w0, w1, w2)]
    bs = [np.asarray(b, dtype=np.float32) for b in (b0, b1, b2)]
    use_bias = any(np.any(b != 0) for b in bs)
    in_maps = _make_in_maps(xs, ws, bs, use_bias)
    res = run_bass_kernel_spmd(_get_prog(use_bias), in_maps,
                               core_ids=list(range(NCORES)), **spmd_kwargs)
    return _assemble(res.results), res


def kernel(x0, x1, x2, w0, b0, w1, b1, w2, b2):
    out, _ = _run(x0, x1, x2, w0, b0, w1, b1, w2, b2)
    return out


def kernel_traced(x0, x1, x2, w0, b0, w1, b1, w2, b2):
    """Like kernel() but with NTFF tracing; returns (out, BassKernelResults)."""
    return _run(x0, x1, x2, w0, b0, w1, b1, w2, b2, trace=True)


# revision 35
# speedup vs baseline: 1.0754x; 1.0754x over previous
"""YOLO-detect head (1x1 conv + box decode) on 8 Trainium2 NeuronCores.

Data-parallel over batch: core b processes batch element b.

Per core, per level l (C channels, HW = ny*nx positions):
  p[hw, o] = sum_c x[c, hw] * w[o, c]      (o = a*89 + ch, a anchor, ch channel)
computed on the tensor engine as out = lhsT.T @ rhs with
  lhsT = x chunk  [K=128 channels, M=128 hw]  (stationary; fp8-e3m4 for levels
         0/1, fp16 for level 2 -- mixed fp8xfp16 matmul is legal on TRN2)
  rhs  = w.T chunk [K=128 channels, N=267]    (moving, fp16)
so the PSUM result is already [hw, 267] -- no on-chip transpose.

Decode (ACT engine is the critical resource: 1 elem/cycle/lane @1.2GHz,
~352cy fixed overhead per ACTIVATE):
  * ONE table set (exp_and_others: tanh+exp) for the whole kernel.
  * Per 4-tile PSUM group, ONE Tanh over all 89 cols writes t = tanh(0.5*p)
    directly as fp8-e3m4 into the big output tile og8.  sigmoid = 0.5*t+0.5
    is applied by the HOST during dequant (a scaled-fp8 codec).
  * wh: DVE stages the raw logits PSUM->SBUF per group (it is otherwise
    idle); ONE Exp per level at level end, then one anchor-multiply.
  * xy: derived per chunk from the fp8 tanh values with fused DVE
    scalar_tensor_tensor ops: xy = t*(stride/2) + (stride*(grid+0.5)).

Scheduling (what v2's trace taught):
  * Levels run L0, then L2 interleaved after L0's 8th group, then L1.
    L2's 32-matmul group fills the PE while ACT drains L0's tanh backlog;
    ending on L1 keeps the serial tail short.
  * Early loads are spread over three HWDGE queues (gsam on vector,
    wt0/wt2/x2 on scalar, x0 pieces + wt1/x1 on sync) so DGE setup times
    overlap and the first matmul starts ~2us sooner.
  * Stores ride nc.gpsimd (SWDGE) so a blocked store never stalls loads.

Error budget (tolerance 2e-2 of absmax~1132): fp8-e3m4 x-quant worst-cases
the level-1 wh channels at ~4e-3 (measured 1.7e-3 total); fp8 t-storage
adds ~7e-6; everything else is at the old fp16 level (~5e-4).
"""

import numpy as np
import ml_dtypes

import concourse.bacc as bacc
import concourse.mybir as mybir
import concourse.tile as tile
from concourse.bass_utils import run_bass_kernel_spmd

F32 = mybir.dt.float32
F16 = mybir.dt.float16
F8 = mybir.dt.float8e3   # e3m4: 4 mantissa bits, range +-15.5
F8E4 = mybir.dt.float8e4  # e4m3 (TRN flavor, max 240) -- DoubleRow needs it
NP_F8 = ml_dtypes.float8_e3m4
NP_F8E4 = ml_dtypes.float8_e4m3
AF = mybir.ActivationFunctionType
ALU = mybir.AluOpType
DR = mybir.MatmulPerfMode.DoubleRow

NCORES = 8
NA = 3          # anchors per level
NO = 89         # channels per anchor (80 classes + 5 + 4)
NCOL = NA * NO  # 267
GROUP = 4       # full 128-row hw tiles per PSUM group (4 banks, 2 in flight)
CHUNK = 16      # tiles per store chunk / per DVE xy-fixup op
WSCALE = 16.0   # pre-quant scale on fp8 w (keeps it out of denormals)
NWARM = 16      # dummy matmuls that trip the HAM clock gate early; 16 x
                # ~432ns cold = ~6.9us busy, covering a full aligned 4096cy
                # HAM window even at the worst free-running phase

LEVELS = [
    dict(C=256,  W=80, HW=6400, stride=8.0, xdt=F8, dr=False, wq=False,
         exp_defer=True,
         anchors=((10.0, 13.0), (16.0, 30.0), (33.0, 23.0))),
    # level 1 runs last: per-group exp keeps its tail short
    dict(C=512,  W=40, HW=1600, stride=16.0, xdt=F8, dr=False, wq=False,
         exp_defer=False,
         anchors=((30.0, 61.0), (62.0, 45.0), (59.0, 119.0))),
    dict(C=1024, W=20, HW=400,  stride=32.0, xdt=F16, dr=False, wq=False,
         exp_defer=False,
         anchors=((116.0, 90.0), (156.0, 198.0), (373.0, 326.0))),
]
for _L in LEVELS:
    _L["nt"] = (_L["HW"] + 127) // 128      # 50, 13, 4
    _L["HWp"] = _L["nt"] * 128              # padded positions (6400, 1664, 512)
    _L["psc"] = WSCALE if _L["wq"] else 1.0  # psum = psc * p

# (level, group-slice) segments in issue order.  L2's PE-heavy group sits
# mid-stream where ACT has backlog; L1 runs last (its final group is 1 tile
# so the tail store chain is tiny).  The PE's queue is in-order, so work
# must not be scheduled before its x data can possibly have arrived.
SCHEDULE = [(0, 0, 9), (2, 0, 1), (0, 9, 10), (2, 1, 2), (0, 10, 12),
            (1, 0, 2), (0, 12, 14), (1, 2, 7)]


def _groups(nt, base=0):
    """[(t0, ntl)] covering tiles [base, base+nt) in GROUP-sized pieces."""
    out = []
    t0 = 0
    while t0 < nt:
        out.append((base + t0, min(GROUP, nt - t0)))
        t0 += GROUP
    return out


def _chunks(nt):
    """[(s0, snt)] store/fixup chunks, group-aligned, trailing runt merged."""
    out = []
    s = 0
    while s < nt:
        e = min(s + CHUNK, nt)
        if nt - e < GROUP:
            e = nt
        out.append((s, e - s))
        s = e
    return out


def _build_program(use_bias: bool):
    nc = bacc.Bacc("TRN2", target_bir_lowering=False, debug=False)

    # gs' and am, both [128, NA, 2, nt] fp16 per level, concatenated
    GSAM_COLS = sum(NA * 2 * L["nt"] * 2 for L in LEVELS)  # 804

    dram = {}
    for l, L in enumerate(LEVELS):
        KC = L["C"] // 128
        nt = L["nt"]
        wdt = F8E4 if L["wq"] else F16
        dram[f"x{l}"] = nc.dram_tensor(f"x{l}", (128, KC * L["HW"]), L["xdt"],
                                       kind="ExternalInput").ap()
        dram[f"wt{l}"] = nc.dram_tensor(f"wt{l}", (128, KC * NCOL), wdt,
                                        kind="ExternalInput").ap()
        # t=tanh(.5 p) for every col; partition p holds hw rows {t*128+p}
        dram[f"y8_{l}"] = nc.dram_tensor(f"y8_{l}", (128, NA, nt, NO), F8,
                                         kind="ExternalOutput").ap()
        # final xy (cols 0:2) and exp(p)*anchor (cols 2:4), c-major runs
        dram[f"yx_{l}"] = nc.dram_tensor(f"yx_{l}", (128, NA, 4, nt), F16,
                                         kind="ExternalOutput").ap()
        if use_bias:
            dram[f"b{l}"] = nc.dram_tensor(f"b{l}", (1, NCOL), F32,
                                           kind="ExternalInput").ap()
    dram["gsam"] = nc.dram_tensor("gsam", (128, GSAM_COLS), F16,
                                  kind="ExternalInput").ap()

    with tile.TileContext(nc) as tc:
        with tc.tile_pool(name="consts", bufs=1) as cpool, \
             tc.tile_pool(name="xbuf", bufs=1) as xpool, \
             tc.tile_pool(name="obuf", bufs=1) as opool, \
             tc.tile_pool(name="ps", bufs=2, space="PSUM") as pspool:

            ones_t = None
            if use_bias:
                ones_t = cpool.tile([1, 128], F16, tag="ones", name="ones")
                nc.vector.memset(ones_t[:, :], 1.0)

            # ---- loads, spread across HWDGE queues ----
            gsam_t = cpool.tile([128, GSAM_COLS], F16, tag="gsam",
                                name="gsamsb")
            nc.gpsimd.dma_start(out=gsam_t[:, :], in_=dram["gsam"][:, :])

            lvl = {}
            for l, L in enumerate(LEVELS):
                KC = L["C"] // 128
                wdt = F8E4 if L["wq"] else F16
                lvl[l] = dict(
                    wt=cpool.tile([128, KC, NCOL], wdt, tag=f"wt{l}",
                                  name=f"wt{l}sb"),
                    xk=xpool.tile([128, KC, L["HWp"]], L["xdt"], tag=f"x{l}",
                                  name=f"xk{l}"),
                    b_t=None,
                )
                if L["HWp"] != L["HW"]:
                    # pad cols feed the partial tile's matmul rows: zero them
                    nc.gpsimd.memset(lvl[l]["xk"][:, :, L["HW"]:L["HWp"]], 0.0)
                if use_bias:
                    lvl[l]["b_t"] = cpool.tile([1, NCOL], F32, tag=f"b{l}",
                                               name=f"bt{l}")
                    nc.gpsimd.dma_start(out=lvl[l]["b_t"][:, :],
                                        in_=dram[f"b{l}"][:, :])

            def xsrc(l):
                return dram[f"x{l}"].rearrange(
                    "p (k w) -> p k w", k=LEVELS[l]["C"] // 128)

            def wsrc(l):
                return dram[f"wt{l}"].rearrange(
                    "p (k o) -> p k o", k=LEVELS[l]["C"] // 128)

            # ALL loads ride the sync queue in strict need-order: a second
            # load queue just steals DMA-engine service from the critical
            # x0 pieces (measured: wt1+wt2 on scalar delayed x0p1 by ~3us)
            nc.sync.dma_start(out=lvl[0]["wt"][:, :, :], in_=wsrc(0))
            for (c0, c1) in ((0, 512), (512, 2560), (2560, 6400)):
                nc.sync.dma_start(out=lvl[0]["xk"][:, :, c0:c1],
                                  in_=xsrc(0)[:, :, c0:c1])
            nc.sync.dma_start(out=lvl[2]["wt"][:, :, :], in_=wsrc(2))
            nc.sync.dma_start(out=lvl[2]["xk"][:, :, 0:400], in_=xsrc(2))
            nc.sync.dma_start(out=lvl[1]["wt"][:, :, :], in_=wsrc(1))
            nc.sync.dma_start(out=lvl[1]["xk"][:, :, 0:1600], in_=xsrc(1))

            off = 0
            for l, L in enumerate(LEVELS):
                n6 = NA * 2 * L["nt"]
                lvl[l]["gs"] = gsam_t[:, off:off + n6].rearrange(
                    "p (a c t) -> p a c t", a=NA, c=2)
                off += n6
                lvl[l]["am"] = gsam_t[:, off:off + n6].rearrange(
                    "p (a c t) -> p a c t", a=NA, c=2)
                off += n6
                lvl[l]["og8"] = opool.tile([128, NA, L["nt"], NO], F8,
                                           tag=f"og8_{l}", name=f"og8_{l}")
                lvl[l]["oxw"] = opool.tile([128, NA, 4, L["nt"]], F16,
                                           tag=f"oxw{l}", name=f"oxw{l}")
                if L["exp_defer"]:
                    lvl[l]["pwh"] = opool.tile([128, NA, L["nt"], 2], F16,
                                               tag=f"pwh{l}", name=f"pwh{l}")
                lvl[l]["chunks"] = ([(0, 8), (8, 5)] if l == 1
                                    else _chunks(L["nt"]))
                lvl[l]["next_chunk"] = 0
                # L2 as two 2-tile groups: its 32-matmul block otherwise
                # starves ACT for ~3us.  L0 starts with two 2-tile groups so
                # the first tanh issues earlier while the PE is still cold.
                if l == 2:
                    lvl[l]["groups"] = [(0, 2), (2, 2)]
                elif l == 0:
                    lvl[l]["groups"] = ([(0, 2), (2, 2)]
                                        + _groups(L["nt"] - 4, base=4))
                else:
                    # 2-tile mini-groups: 8 matmuls (~1.2us) vs tanh+exp
                    # (~1.0us) self-balance; 4-tile groups are PE-bound
                    lvl[l]["groups"] = [(t, min(2, L["nt"] - t))
                                        for t in range(0, L["nt"], 2)]

            # ---- compute; stores via SWDGE (gpsimd) ----
            def emit_segment(l, g0, g1):
                L = LEVELS[l]
                KC = L["C"] // 128
                stride, nt, psc = L["stride"], L["nt"], L["psc"]
                d = lvl[l]
                wt_t, xk, b_t = d["wt"], d["xk"], d["b_t"]
                og8, oxw = d["og8"], d["oxw"]

                for (t0, ntl) in d["groups"][g0:g1]:
                    ps = pspool.tile([128, GROUP, 512], F32, tag="ps",
                                     name=f"ps{l}_{t0}")
                    psf = ps.rearrange("p g x -> p (g x)")
                    for i in range(ntl):
                        t = t0 + i
                        if L["dr"]:
                            # fp8 DoubleRow: k-pairs as [Ki, 2, free] APs
                            for kc in range(0, KC, 2):
                                nc.tensor.matmul(
                                    psf[:, i * 512:i * 512 + NCOL],
                                    lhsT=xk[:, kc:kc + 2,
                                            t * 128:(t + 1) * 128],
                                    rhs=wt_t[:, kc:kc + 2, :],
                                    start=(kc == 0),
                                    stop=(kc == KC - 2 and not use_bias),
                                    perf_mode=DR,
                                )
                        else:
                            for kc in range(KC):
                                nc.tensor.matmul(
                                    psf[:, i * 512:i * 512 + NCOL],
                                    lhsT=xk[:, kc, t * 128:(t + 1) * 128],
                                    rhs=wt_t[:, kc, :],
                                    start=(kc == 0),
                                    stop=(kc == KC - 1 and not use_bias),
                                )
                        if use_bias:
                            nc.tensor.matmul(
                                psf[:, i * 512:i * 512 + NCOL],
                                lhsT=ones_t[:, :],
                                rhs=b_t[:, :],
                                start=False,
                                stop=True,
                            )

                    # psum viewed anchor-major: [p, a, g, c]; psum = psc * p
                    ps_a = ps[:, 0:ntl, 0:NCOL].rearrange(
                        "p g (a c) -> p a g c", a=NA)
                    # t = tanh(0.5*p) straight to fp8; host decodes 0.5t+0.5
                    nc.scalar.activation(og8[:, :, t0:t0 + ntl, :], ps_a,
                                         AF.Tanh, scale=0.5 / psc)
                    if L["exp_defer"]:
                        # stage wh logits for the per-level batched Exp
                        nc.vector.tensor_copy(d["pwh"][:, :, t0:t0 + ntl, :],
                                              ps_a[:, :, :, 2:4])
                    else:
                        nc.scalar.activation(
                            oxw[:, :, 2:4, t0:t0 + ntl],
                            ps_a[:, :, :, 2:4].transpose([0, 1, 3, 2]),
                            AF.Exp, scale=1.0 / psc)

                    while (d["next_chunk"] < len(d["chunks"])
                           and d["chunks"][d["next_chunk"]][0]
                           + d["chunks"][d["next_chunk"]][1] <= t0 + ntl):
                        s0, snt = d["chunks"][d["next_chunk"]]
                        # xy = t*(stride/2) + stride*(grid+0.5)
                        # (fused stt; one per anchor -- stt APs max 3D)
                        for a in range(NA):
                            nc.vector.scalar_tensor_tensor(
                                oxw[:, a, 0:2, s0:s0 + snt],
                                og8[:, a, s0:s0 + snt, 0:2]
                                .transpose([0, 2, 1]),
                                float(stride / 2),
                                d["gs"][:, a, :, s0:s0 + snt],
                                ALU.mult, ALU.add)
                        final = (l == 1 and d["next_chunk"]
                                 == len(d["chunks"]) - 1)
                        # route the kernel's last og8 store via the (idle by
                        # then) sync HWDGE so it issues in parallel with the
                        # oxw store below
                        eng = nc.sync if final else nc.gpsimd
                        eng.dma_start(
                            out=dram[f"y8_{l}"][:, :, s0:s0 + snt, :],
                            in_=og8[:, :, s0:s0 + snt, :])
                        d["next_chunk"] += 1

                if g1 >= len(d["groups"]):  # level finished
                    assert d["next_chunk"] == len(d["chunks"])
                    if L["exp_defer"]:
                        # wh = exp(p): one batched Exp off the staged logits
                        nc.scalar.activation(
                            oxw[:, :, 2:4, :],
                            d["pwh"][:, :, :, :].transpose([0, 1, 3, 2]),
                            AF.Exp, scale=1.0 / psc)
                    nc.vector.tensor_mul(oxw[:, :, 2:4, :],
                                         oxw[:, :, 2:4, :],
                                         d["am"][:, :, :, :])
                    eng = nc.scalar if l == 1 else nc.gpsimd
                    eng.dma_start(out=dram[f"yx_{l}"][:, :, :, :],
                                  in_=oxw[:, :, :, :])

            for (l, g0, g1) in SCHEDULE:
                emit_segment(l, g0, g1)
    nc.compile()
    return nc


_PROGS = {}


def _get_prog(use_bias: bool):
    if use_bias not in _PROGS:
        _PROGS[use_bias] = _build_program(use_bias)
    return _PROGS[use_bias]


def _host_gsam():
    """[gs'0|am0|gs'1|am1|gs'2|am2], each [128, NA, 2, nt] fp16 flattened.

    gs'[p, a, c, t] = stride*(grid_c(t*128+p) + 0.5); am[p, a, c, t] = A[a][c].
    """
    cols = []
    for L in LEVELS:
        HW, W, stride, nt = L["HW"], L["W"], L["stride"], L["nt"]
        hw = np.arange(nt * 128)
        gx = (hw % W).astype(np.float32)
        gy = (hw // W).astype(np.float32)
        g = np.stack([gx, gy], axis=0)          # (2, nt*128)
        gsp = (g + 0.5) * stride
        gsp[:, HW:] = 0.0
        # (2, nt, 128) -> [p, c, t]
        gsp = gsp.reshape(2, nt, 128).transpose(2, 0, 1)
        gs = np.broadcast_to(gsp[:, None], (128, NA, 2, nt))
        am = np.broadcast_to(
            np.asarray(L["anchors"], np.float32)[None, :, :, None],
            (128, NA, 2, nt))
        cols.append(gs.reshape(128, -1))
        cols.append(am.reshape(128, -1))
    return np.ascontiguousarray(
        np.concatenate(cols, axis=1).astype(np.float16))


_CONSTS = None


def _make_in_maps(xs, ws, bs, use_bias):
    global _CONSTS
    if _CONSTS is None:
        _CONSTS = _host_gsam()
    wts, xps = [], []
    for x, w, L in zip(xs, ws, LEVELS):
        KC = L["C"] // 128
        HW = L["HW"]
        npdt = {F8: NP_F8, F8E4: NP_F8E4, F16: np.float16}[L["xdt"]]
        wdt = NP_F8E4 if L["wq"] else np.float16
        # (C, NCOL) -> (128, KC*NCOL): row p col (k*NCOL+o) = w[o, k*128+p]
        wts.append(np.ascontiguousarray(
            (w.T * (WSCALE if L["wq"] else 1.0)).astype(wdt)
            .reshape(KC, 128, NCOL)
            .transpose(1, 0, 2).reshape(128, KC * NCOL)))
        # (B, C, H, W) -> (B, 128, KC*HW): row p col (k*HW+hw) = x[k*128+p, hw]
        xps.append(np.ascontiguousarray(
            x.reshape(NCORES, KC, 128, HW).astype(npdt)
            .transpose(0, 2, 1, 3).reshape(NCORES, 128, KC * HW)))
    in_maps = []
    for core in range(NCORES):
        im = {"gsam": _CONSTS}
        for l in range(len(LEVELS)):
            im[f"x{l}"] = xps[l][core]
            im[f"wt{l}"] = wts[l]
            if use_bias:
                im[f"b{l}"] = np.ascontiguousarray(
                    (bs[l] * LEVELS[l]["psc"]).reshape(1, NCOL)
                    .astype(np.float32))
        in_maps.append(im)
    return in_maps


def _assemble(results):
    """y8 (128,NA,nt,89) fp8 + yx (128,NA,4,nt) fp16 -> (NCORES,25200,89)."""
    out = np.empty((NCORES, 25200, NO), np.float32)
    for core in range(NCORES):
        parts = []
        for l, L in enumerate(LEVELS):
            HW, nt = L["HW"], L["nt"]
            t8 = results[core][f"y8_{l}"].astype(np.float32)
            # sigmoid = 0.5*t + 0.5 (fp8 codec dequant)
            y = t8 * 0.5 + 0.5
            y = y.transpose(1, 2, 0, 3).reshape(NA, nt * 128, NO)[:, :HW, :]
            xw = results[core][f"yx_{l}"].astype(np.float32)
            xw = xw.transpose(1, 3, 0, 2).reshape(NA, nt * 128, 4)[:, :HW, :]
            y[:, :, 0:4] = xw
            parts.append(y.reshape(NA * HW, NO))
        out[core] = np.concatenate(parts, axis=0)
    return out


def _run(x0, x1, x2, w0, b0, w1, b1, w2, b2, **spmd_kwargs):
    xs = [np.asarray(x, dtype=np.float32) for x in (x0, x1, x2)]
    ws = [np.asarray(w, dtype=np.float32) for w in (w0, w1, w2)]
    bs = [np.asarray(b, dtype=np.float32) for b in (b0, b1, b2)]
    use_bias = any(np.any(b != 0) for b in bs)
    in_maps = _make_in_maps(xs, ws, bs, use_bias)
    res = run_bass_kernel_spmd(_get_prog(use_bias), in_maps,
                               core_ids=list(range(NCORES)), **spmd_kwargs)
    return _assemble(res.results), res


def kernel(x0, x1, x2, w0, b0, w1, b1, w2, b2):
    out, _ = _run(x0, x1, x2, w0, b0, w1, b1, w2, b2)
    return out


def kernel_traced(x0, x1, x2, w0, b0, w1, b1, w2, b2):
    """Like kernel() but with NTFF tracing; returns (out, BassKernelResults)."""
    return _run(x0, x1, x2, w0, b0, w1, b1, w2, b2, trace=True)


# revision 39
# speedup vs baseline: 1.1726x; 1.0904x over previous
"""YOLO-detect head (1x1 conv + box decode) on 8 Trainium2 NeuronCores.

Data-parallel over batch: core b processes batch element b.

Per core, per level l (C channels, HW = ny*nx positions):
  p[hw, o] = sum_c x[c, hw] * w[o, c]      (o = a*89 + ch, a anchor, ch channel)
computed on the tensor engine as out = lhsT.T @ rhs with
  lhsT = x chunk  [K=128 channels, M=128 hw]  (stationary; fp8-e3m4 for levels
         0/1, fp16 for level 2 -- mixed fp8xfp16 matmul is legal on TRN2)
  rhs  = w.T chunk [K=128 channels, N=267]    (moving, fp16)
so the PSUM result is already [hw, 267] -- no on-chip transpose.

Decode (ACT engine is the critical resource: 1 elem/cycle/lane @1.2GHz,
~352cy fixed overhead per ACTIVATE):
  * ONE table set (exp_and_others: tanh+exp) for the whole kernel.
  * Per 4-tile PSUM group, ONE Tanh over all 89 cols writes t = tanh(0.5*p)
    directly as fp8-e3m4 into the big output tile og8.  sigmoid = 0.5*t+0.5
    is applied by the HOST during dequant (a scaled-fp8 codec).
  * wh: DVE stages the raw logits PSUM->SBUF per group (it is otherwise
    idle); ONE Exp per level at level end, then one anchor-multiply.
  * xy: derived per chunk from the fp8 tanh values with fused DVE
    scalar_tensor_tensor ops: xy = t*(stride/2) + (stride*(grid+0.5)).

Scheduling (what v2's trace taught):
  * Levels run L0, then L2 interleaved after L0's 8th group, then L1.
    L2's 32-matmul group fills the PE while ACT drains L0's tanh backlog;
    ending on L1 keeps the serial tail short.
  * Early loads are spread over three HWDGE queues (gsam on vector,
    wt0/wt2/x2 on scalar, x0 pieces + wt1/x1 on sync) so DGE setup times
    overlap and the first matmul starts ~2us sooner.
  * Stores ride nc.gpsimd (SWDGE) so a blocked store never stalls loads.

Error budget (tolerance 2e-2 of absmax~1132): fp8-e3m4 x-quant worst-cases
the level-1 wh channels at ~4e-3 (measured 1.7e-3 total); fp8 t-storage
adds ~7e-6; everything else is at the old fp16 level (~5e-4).
"""

import numpy as np
import ml_dtypes

import concourse.bacc as bacc
import concourse.mybir as mybir
import concourse.tile as tile
from concourse.bass_utils import run_bass_kernel_spmd

F32 = mybir.dt.float32
F16 = mybir.dt.float16
F8 = mybir.dt.float8e3   # e3m4: 4 mantissa bits, range +-15.5
F8E4 = mybir.dt.float8e4  # e4m3 (TRN flavor, max 240) -- DoubleRow needs it
NP_F8 = ml_dtypes.float8_e3m4
NP_F8E4 = ml_dtypes.float8_e4m3
AF = mybir.ActivationFunctionType
ALU = mybir.AluOpType
DR = mybir.MatmulPerfMode.DoubleRow

NCORES = 8
NA = 3          # anchors per level
NO = 89         # channels per anchor (80 classes + 5 + 4)
NCOL = NA * NO  # 267
GROUP = 2       # full 128-row hw tiles per PSUM group (2 banks, 4 in flight
                # -- the deep rotation lets ACT accumulate backlog that
                # absorbs the PE-heavy L1/L2 matmul bursts)
CHUNK = 16      # tiles per store chunk / per DVE xy-fixup op
WSCALE = 16.0   # pre-quant scale on fp8 w (keeps it out of denormals)
NWARM = 16      # dummy matmuls that trip the HAM clock gate early; 16 x
                # ~432ns cold = ~6.9us busy, covering a full aligned 4096cy
                # HAM window even at the worst free-running phase

LEVELS = [
    dict(C=256,  W=80, HW=6400, stride=8.0, xdt=F8, dr=False, wq=False,
         exp_defer=True,
         anchors=((10.0, 13.0), (16.0, 30.0), (33.0, 23.0))),
    # level 1 runs last: per-group exp keeps its tail short
    dict(C=512,  W=40, HW=1600, stride=16.0, xdt=F8, dr=False, wq=False,
         exp_defer=False,
         anchors=((30.0, 61.0), (62.0, 45.0), (59.0, 119.0))),
    dict(C=1024, W=20, HW=400,  stride=32.0, xdt=F16, dr=False, wq=False,
         exp_defer=False,
         anchors=((116.0, 90.0), (156.0, 198.0), (373.0, 326.0))),
]
for _L in LEVELS:
    _L["nt"] = (_L["HW"] + 127) // 128      # 50, 13, 4
    _L["HWp"] = _L["nt"] * 128              # padded positions (6400, 1664, 512)
    _L["psc"] = WSCALE if _L["wq"] else 1.0  # psum = psc * p

# (level, group-slice) segments in issue order.  L2's PE-heavy group sits
# mid-stream where ACT has backlog; L1 runs last (its final group is 1 tile
# so the tail store chain is tiny).  The PE's queue is in-order, so work
# must not be scheduled before its x data can possibly have arrived.
SCHEDULE = [(0, 0, 16), (2, 0, 1), (0, 16, 17), (2, 1, 2), (0, 17, 19),
            (1, 0, 2), (0, 19, 25), (1, 2, 7)]


def _groups(nt, base=0):
    """[(t0, ntl)] covering tiles [base, base+nt) in GROUP-sized pieces."""
    out = []
    t0 = 0
    while t0 < nt:
        out.append((base + t0, min(GROUP, nt - t0)))
        t0 += GROUP
    return out


def _chunks(nt):
    """[(s0, snt)] store/fixup chunks, group-aligned, trailing runt merged."""
    out = []
    s = 0
    while s < nt:
        e = min(s + CHUNK, nt)
        if nt - e < GROUP:
            e = nt
        out.append((s, e - s))
        s = e
    return out


def _build_program(use_bias: bool):
    nc = bacc.Bacc("TRN2", target_bir_lowering=False, debug=False)

    # gs' and am, both [128, NA, 2, nt] fp16 per level, concatenated
    GSAM_COLS = sum(NA * 2 * L["nt"] * 2 for L in LEVELS)  # 804

    dram = {}
    for l, L in enumerate(LEVELS):
        KC = L["C"] // 128
        nt = L["nt"]
        wdt = F8E4 if L["wq"] else F16
        dram[f"x{l}"] = nc.dram_tensor(f"x{l}", (128, KC * L["HW"]), L["xdt"],
                                       kind="ExternalInput").ap()
        dram[f"wt{l}"] = nc.dram_tensor(f"wt{l}", (128, KC * NCOL), wdt,
                                        kind="ExternalInput").ap()
        # t=tanh(.5 p) for every col; partition p holds hw rows {t*128+p}
        dram[f"y8_{l}"] = nc.dram_tensor(f"y8_{l}", (128, NA, nt, NO), F8,
                                         kind="ExternalOutput").ap()
        # final xy (cols 0:2) and exp(p)*anchor (cols 2:4), c-major runs
        dram[f"yx_{l}"] = nc.dram_tensor(f"yx_{l}", (128, NA, 4, nt), F16,
                                         kind="ExternalOutput").ap()
        if use_bias:
            dram[f"b{l}"] = nc.dram_tensor(f"b{l}", (1, NCOL), F32,
                                           kind="ExternalInput").ap()
    dram["gsam"] = nc.dram_tensor("gsam", (128, GSAM_COLS), F16,
                                  kind="ExternalInput").ap()

    with tile.TileContext(nc) as tc:
        with tc.tile_pool(name="consts", bufs=1) as cpool, \
             tc.tile_pool(name="xbuf", bufs=1) as xpool, \
             tc.tile_pool(name="obuf", bufs=1) as opool, \
             tc.tile_pool(name="ps", bufs=4, space="PSUM") as pspool:

            ones_t = None
            if use_bias:
                ones_t = cpool.tile([1, 128], F16, tag="ones", name="ones")
                nc.vector.memset(ones_t[:, :], 1.0)

            # ---- loads, spread across HWDGE queues ----
            gsam_t = cpool.tile([128, GSAM_COLS], F16, tag="gsam",
                                name="gsamsb")
            nc.gpsimd.dma_start(out=gsam_t[:, :], in_=dram["gsam"][:, :])

            lvl = {}
            for l, L in enumerate(LEVELS):
                KC = L["C"] // 128
                wdt = F8E4 if L["wq"] else F16
                lvl[l] = dict(
                    wt=cpool.tile([128, KC, NCOL], wdt, tag=f"wt{l}",
                                  name=f"wt{l}sb"),
                    xk=xpool.tile([128, KC, L["HWp"]], L["xdt"], tag=f"x{l}",
                                  name=f"xk{l}"),
                    b_t=None,
                )
                if L["HWp"] != L["HW"]:
                    # pad cols feed the partial tile's matmul rows: zero them
                    nc.gpsimd.memset(lvl[l]["xk"][:, :, L["HW"]:L["HWp"]], 0.0)
                if use_bias:
                    lvl[l]["b_t"] = cpool.tile([1, NCOL], F32, tag=f"b{l}",
                                               name=f"bt{l}")
                    nc.gpsimd.dma_start(out=lvl[l]["b_t"][:, :],
                                        in_=dram[f"b{l}"][:, :])

            def xsrc(l):
                return dram[f"x{l}"].rearrange(
                    "p (k w) -> p k w", k=LEVELS[l]["C"] // 128)

            def wsrc(l):
                return dram[f"wt{l}"].rearrange(
                    "p (k o) -> p k o", k=LEVELS[l]["C"] // 128)

            # ALL loads ride the sync queue in strict need-order: a second
            # load queue just steals DMA-engine service from the critical
            # x0 pieces (measured: wt1+wt2 on scalar delayed x0p1 by ~3us)
            nc.sync.dma_start(out=lvl[0]["wt"][:, :, :], in_=wsrc(0))
            for (c0, c1) in ((0, 512), (512, 2560), (2560, 6400)):
                nc.sync.dma_start(out=lvl[0]["xk"][:, :, c0:c1],
                                  in_=xsrc(0)[:, :, c0:c1])
            nc.sync.dma_start(out=lvl[2]["wt"][:, :, :], in_=wsrc(2))
            nc.sync.dma_start(out=lvl[2]["xk"][:, :, 0:400], in_=xsrc(2))
            nc.sync.dma_start(out=lvl[1]["wt"][:, :, :], in_=wsrc(1))
            nc.sync.dma_start(out=lvl[1]["xk"][:, :, 0:1600], in_=xsrc(1))

            off = 0
            for l, L in enumerate(LEVELS):
                n6 = NA * 2 * L["nt"]
                lvl[l]["gs"] = gsam_t[:, off:off + n6].rearrange(
                    "p (a c t) -> p a c t", a=NA, c=2)
                off += n6
                lvl[l]["am"] = gsam_t[:, off:off + n6].rearrange(
                    "p (a c t) -> p a c t", a=NA, c=2)
                off += n6
                lvl[l]["og8"] = opool.tile([128, NA, L["nt"], NO], F8,
                                           tag=f"og8_{l}", name=f"og8_{l}")
                lvl[l]["oxw"] = opool.tile([128, NA, 4, L["nt"]], F16,
                                           tag=f"oxw{l}", name=f"oxw{l}")
                if L["exp_defer"]:
                    lvl[l]["pwh"] = opool.tile([128, NA, L["nt"], 2], F16,
                                               tag=f"pwh{l}", name=f"pwh{l}")
                lvl[l]["chunks"] = ([(0, 8), (8, 5)] if l == 1
                                    else _chunks(L["nt"]))
                lvl[l]["next_chunk"] = 0
                # L2 as two 2-tile groups: its 32-matmul block otherwise
                # starves ACT for ~3us.  L0 starts with two 2-tile groups so
                # the first tanh issues earlier while the PE is still cold.
                lvl[l]["groups"] = _groups(L["nt"])

            # ---- compute; stores via SWDGE (gpsimd) ----
            def emit_segment(l, g0, g1):
                L = LEVELS[l]
                KC = L["C"] // 128
                stride, nt, psc = L["stride"], L["nt"], L["psc"]
                d = lvl[l]
                wt_t, xk, b_t = d["wt"], d["xk"], d["b_t"]
                og8, oxw = d["og8"], d["oxw"]

                for (t0, ntl) in d["groups"][g0:g1]:
                    ps = pspool.tile([128, GROUP, 512], F32, tag="ps",
                                     name=f"ps{l}_{t0}")
                    psf = ps.rearrange("p g x -> p (g x)")
                    for i in range(ntl):
                        t = t0 + i
                        if L["dr"]:
                            # fp8 DoubleRow: k-pairs as [Ki, 2, free] APs
                            for kc in range(0, KC, 2):
                                nc.tensor.matmul(
                                    psf[:, i * 512:i * 512 + NCOL],
                                    lhsT=xk[:, kc:kc + 2,
                                            t * 128:(t + 1) * 128],
                                    rhs=wt_t[:, kc:kc + 2, :],
                                    start=(kc == 0),
                                    stop=(kc == KC - 2 and not use_bias),
                                    perf_mode=DR,
                                )
                        else:
                            for kc in range(KC):
                                nc.tensor.matmul(
                                    psf[:, i * 512:i * 512 + NCOL],
                                    lhsT=xk[:, kc, t * 128:(t + 1) * 128],
                                    rhs=wt_t[:, kc, :],
                                    start=(kc == 0),
                                    stop=(kc == KC - 1 and not use_bias),
                                )
                        if use_bias:
                            nc.tensor.matmul(
                                psf[:, i * 512:i * 512 + NCOL],
                                lhsT=ones_t[:, :],
                                rhs=b_t[:, :],
                                start=False,
                                stop=True,
                            )

                    # psum viewed anchor-major: [p, a, g, c]; psum = psc * p
                    ps_a = ps[:, 0:ntl, 0:NCOL].rearrange(
                        "p g (a c) -> p a g c", a=NA)
                    # t = tanh(0.5*p) straight to fp8; host decodes 0.5t+0.5
                    nc.scalar.activation(og8[:, :, t0:t0 + ntl, :], ps_a,
                                         AF.Tanh, scale=0.5 / psc)
                    if L["exp_defer"]:
                        # stage wh logits for the per-level batched Exp
                        nc.vector.tensor_copy(d["pwh"][:, :, t0:t0 + ntl, :],
                                              ps_a[:, :, :, 2:4])
                    else:
                        nc.scalar.activation(
                            oxw[:, :, 2:4, t0:t0 + ntl],
                            ps_a[:, :, :, 2:4].transpose([0, 1, 3, 2]),
                            AF.Exp, scale=1.0 / psc)

                    while (d["next_chunk"] < len(d["chunks"])
                           and d["chunks"][d["next_chunk"]][0]
                           + d["chunks"][d["next_chunk"]][1] <= t0 + ntl):
                        s0, snt = d["chunks"][d["next_chunk"]]
                        # xy = t*(stride/2) + stride*(grid+0.5)
                        # (fused stt; one per anchor -- stt APs max 3D)
                        for a in range(NA):
                            nc.vector.scalar_tensor_tensor(
                                oxw[:, a, 0:2, s0:s0 + snt],
                                og8[:, a, s0:s0 + snt, 0:2]
                                .transpose([0, 2, 1]),
                                float(stride / 2),
                                d["gs"][:, a, :, s0:s0 + snt],
                                ALU.mult, ALU.add)
                        final = (l == 1 and d["next_chunk"]
                                 == len(d["chunks"]) - 1)
                        # route the kernel's last og8 store via the (idle by
                        # then) sync HWDGE so it issues in parallel with the
                        # oxw store below
                        eng = nc.sync if final else nc.gpsimd
                        eng.dma_start(
                            out=dram[f"y8_{l}"][:, :, s0:s0 + snt, :],
                            in_=og8[:, :, s0:s0 + snt, :])
                        d["next_chunk"] += 1

                if g1 >= len(d["groups"]):  # level finished
                    assert d["next_chunk"] == len(d["chunks"])
                    if L["exp_defer"]:
                        # wh = exp(p): one batched Exp off the staged logits
                        nc.scalar.activation(
                            oxw[:, :, 2:4, :],
                            d["pwh"][:, :, :, :].transpose([0, 1, 3, 2]),
                            AF.Exp, scale=1.0 / psc)
                    nc.vector.tensor_mul(oxw[:, :, 2:4, :],
                                         oxw[:, :, 2:4, :],
                                         d["am"][:, :, :, :])
                    eng = nc.scalar if l == 1 else nc.gpsimd
                    eng.dma_start(out=dram[f"yx_{l}"][:, :, :, :],
                                  in_=oxw[:, :, :, :])

            for (l, g0, g1) in SCHEDULE:
                emit_segment(l, g0, g1)
    nc.compile()
    return nc


_PROGS = {}


def _get_prog(use_bias: bool):
    if use_bias not in _PROGS:
        _PROGS[use_bias] = _build_program(use_bias)
    return _PROGS[use_bias]


def _host_gsam():
    """[gs'0|am0|gs'1|am1|gs'2|am2], each [128, NA, 2, nt] fp16 flattened.

    gs'[p, a, c, t] = stride*(grid_c(t*128+p) + 0.5); am[p, a, c, t] = A[a][c].
    """
    cols = []
    for L in LEVELS:
        HW, W, stride, nt = L["HW"], L["W"], L["stride"], L["nt"]
        hw = np.arange(nt * 128)
        gx = (hw % W).astype(np.float32)
        gy = (hw // W).astype(np.float32)
        g = np.stack([gx, gy], axis=0)          # (2, nt*128)
        gsp = (g + 0.5) * stride
        gsp[:, HW:] = 0.0
        # (2, nt, 128) -> [p, c, t]
        gsp = gsp.reshape(2, nt, 128).transpose(2, 0, 1)
        gs = np.broadcast_to(gsp[:, None], (128, NA, 2, nt))
        am = np.broadcast_to(
            np.asarray(L["anchors"], np.float32)[None, :, :, None],
            (128, NA, 2, nt))
        cols.append(gs.reshape(128, -1))
        cols.append(am.reshape(128, -1))
    return np.ascontiguousarray(
        np.concatenate(cols, axis=1).astype(np.float16))


_CONSTS = None


def _make_in_maps(xs, ws, bs, use_bias):
    global _CONSTS
    if _CONSTS is None:
        _CONSTS = _host_gsam()
    wts, xps = [], []
    for x, w, L in zip(xs, ws, LEVELS):
        KC = L["C"] // 128
        HW = L["HW"]
        npdt = {F8: NP_F8, F8E4: NP_F8E4, F16: np.float16}[L["xdt"]]
        wdt = NP_F8E4 if L["wq"] else np.float16
        # (C, NCOL) -> (128, KC*NCOL): row p col (k*NCOL+o) = w[o, k*128+p]
        wts.append(np.ascontiguousarray(
            (w.T * (WSCALE if L["wq"] else 1.0)).astype(wdt)
            .reshape(KC, 128, NCOL)
            .transpose(1, 0, 2).reshape(128, KC * NCOL)))
        # (B, C, H, W) -> (B, 128, KC*HW): row p col (k*HW+hw) = x[k*128+p, hw]
        xps.append(np.ascontiguousarray(
            x.reshape(NCORES, KC, 128, HW).astype(npdt)
            .transpose(0, 2, 1, 3).reshape(NCORES, 128, KC * HW)))
    in_maps = []
    for core in range(NCORES):
        im = {"gsam": _CONSTS}
        for l in range(len(LEVELS)):
            im[f"x{l}"] = xps[l][core]
            im[f"wt{l}"] = wts[l]
            if use_bias:
                im[f"b{l}"] = np.ascontiguousarray(
                    (bs[l] * LEVELS[l]["psc"]).reshape(1, NCOL)
                    .astype(np.float32))
        in_maps.append(im)
    return in_maps


def _assemble(results):
    """y8 (128,NA,nt,89) fp8 + yx (128,NA,4,nt) fp16 -> (NCORES,25200,89)."""
    out = np.empty((NCORES, 25200, NO), np.float32)
    for core in range(NCORES):
        parts = []
        for l, L in enumerate(LEVELS):
            HW, nt = L["HW"], L["nt"]
            t8 = results[core][f"y8_{l}"].astype(np.float32)
            # sigmoid = 0.5*t + 0.5 (fp8 codec dequant)
            y = t8 * 0.5 + 0.5
            y = y.transpose(1, 2, 0, 3).reshape(NA, nt * 128, NO)[:, :HW, :]
            xw = results[core][f"yx_{l}"].astype(np.float32)
            xw = xw.transpose(1, 3, 0, 2).reshape(NA, nt * 128, 4)[:, :HW, :]
            y[:, :, 0:4] = xw
            parts.append(y.reshape(NA * HW, NO))
        out[core] = np.concatenate(parts, axis=0)
    return out


def _run(x0, x1, x2, w0, b0, w1, b1, w2, b2, **spmd_kwargs):
    xs = [np.asarray(x, dtype=np.float32) for x in (x0, x1, x2)]
    ws = [np.asarray(w, dtype=np.float32) for w in (w0, w1, w2)]
    bs = [np.asarray(b, dtype=np.float32) for b in (b0, b1, b2)]
    use_bias = any(np.any(b != 0) for b in bs)
    in_maps = _make_in_maps(xs, ws, bs, use_bias)
    res = run_bass_kernel_spmd(_get_prog(use_bias), in_maps,
                               core_ids=list(range(NCORES)), **spmd_kwargs)
    return _assemble(res.results), res


def kernel(x0, x1, x2, w0, b0, w1, b1, w2, b2):
    out, _ = _run(x0, x1, x2, w0, b0, w1, b1, w2, b2)
    return out


def kernel_traced(x0, x1, x2, w0, b0, w1, b1, w2, b2):
    """Like kernel() but with NTFF tracing; returns (out, BassKernelResults)."""
    return _run(x0, x1, x2, w0, b0, w1, b1, w2, b2, trace=True)
